# revision 15
# baseline (speedup 1.0000x reference)
"""Block-causal self-attention on 8 Trainium2 NeuronCores.

Sharding: data-parallel over batch (B=8 -> one batch element per core).
Weights replicated. No collectives.

Per-core Bass program (all fp32 storage, fp32r matmuls):
  - inputs arrive pre-transposed on host: xT=[C,T], w*T=[C,C] (c-major)
  - qT,kT = W @ xT + b   (feature-on-partition layout [C,T])
  - v     = x @ WvT + bv (natural [T,C]) stored per-head with a ones column
  - per (head, query-pair): scores^T = k_tile^T.T @ q  -> exp -> AV matmul
    (v ones column accumulates the softmax denominator as an extra row)
  - y_norm = y_u * replicate(1/l); out = y_norm^T proj + bp (rank-1 bias)
"""

import contextlib
import math

import numpy as np

import concourse.bass as bass
import concourse.mybir as mybir
import concourse.tile as tile
from concourse import bacc
from concourse.bass_utils import run_bass_kernel_spmd

F32 = mybir.dt.float32
F32R = mybir.dt.float32r
EXP = mybir.ActivationFunctionType.Exp
IDENT = mybir.ActivationFunctionType.Identity

B, T, C = 8, 1024, 512
H = 8
D = C // H          # 64
NF = 128            # frames
NA = 8              # animals per frame
NT = T // 128       # 8 query/key tiles of 128
NC4 = C // 128      # 4 feature tiles
NEG = -1e9


def build_attention(tc, out_ap, ins, general_mask):
    """Emit the per-core attention program into TileContext tc.

    ins: dict of input APs (DRAM).
    """
    nc = tc.nc
    xT, wqT, wkT, wvT, wpT = ins["xT"], ins["wqT"], ins["wkT"], ins["wvT"], ins["wpT"]
    bq_t, bk_t = ins["bq_t"], ins["bk_t"]
    bv_row, bp_row = ins["bv_row"], ins["bp_row"]
    m01 = ins["m01"]
    ones_in = ins["ones_in"]
    kmask = ins.get("kmask")

    # ---------------- persistent SBUF tiles ----------------
    frees = []

    def t_sb(name, shape, dtype=F32R):
        tl, free = tc.tile(shape, dtype, name=name)
        frees.append(free)
        return tl

    xt = [t_sb(f"xt{i}", [128, T]) for i in range(NC4)]
    wq = [t_sb(f"wq{i}", [128, C]) for i in range(NC4)]
    wk = [t_sb(f"wk{i}", [128, C]) for i in range(NC4)]
    wv = [t_sb(f"wv{i}", [128, C]) for i in range(NC4)]
    wp = [t_sb(f"wp{i}", [128, C]) for i in range(NC4)]
    qT = [t_sb(f"qT{i}", [128, T]) for i in range(NC4)]
    kT = [t_sb(f"kT{i}", [128, T]) for i in range(NC4)]
    # v per t-tile: [128, head, 65]; even head h: v cols 0:64, ones col 64
    #               odd  head h: ones col 0, v cols 1:65
    vt = [t_sb(f"vt{i}", [128, H, 66]) for i in range(NT)]
    yT = [t_sb(f"yT{i}", [128, T]) for i in range(NC4)]
    ones_sb = t_sb("ones_sb", [128, 128])
    m01_sb = t_sb("m01_sb", [128, 512], F32)
    bq_sb = t_sb("bq_sb", [128, NC4], F32)
    bk_sb = t_sb("bk_sb", [128, NC4], F32)
    bv_sb = t_sb("bv_sb", [1, C])
    bp_sb = t_sb("bp_sb", [1, C])
    km_sb = t_sb("km_sb", [128, NT], F32) if general_mask else None

    # ---------------- input DMAs ----------------
    for i in range(NC4):
        nc.sync.dma_start(out=xt[i], in_=xT[i * 128:(i + 1) * 128, :].bitcast(F32R))
        nc.sync.dma_start(out=wq[i], in_=wqT[i * 128:(i + 1) * 128, :].bitcast(F32R))
        nc.sync.dma_start(out=wk[i], in_=wkT[i * 128:(i + 1) * 128, :].bitcast(F32R))
        nc.sync.dma_start(out=wv[i], in_=wvT[i * 128:(i + 1) * 128, :].bitcast(F32R))
        nc.sync.dma_start(out=wp[i], in_=wpT[i * 128:(i + 1) * 128, :].bitcast(F32R))
    nc.sync.dma_start(out=m01_sb, in_=m01)
    nc.sync.dma_start(out=bq_sb, in_=bq_t)
    nc.sync.dma_start(out=bk_sb, in_=bk_t)
    nc.sync.dma_start(out=bv_sb, in_=bv_row.bitcast(F32R))
    nc.sync.dma_start(out=bp_sb, in_=bp_row.bitcast(F32R))
    if general_mask:
        nc.sync.dma_start(out=km_sb, in_=kmask)
    nc.sync.dma_start(out=ones_sb, in_=ones_in.bitcast(F32R))
    for i in range(NT):
        # ones cols (softmax denominator rows in the AV matmul)
        nc.sync.dma_start(out=vt[i][:, :, 64:66],
                          in_=ones_in[:, 0:16].rearrange("p (h o) -> p h o", h=H).bitcast(F32R))

    # ---------------- pools ----------------
    ctx = contextlib.ExitStack()
    with ctx:
        mm_pool = ctx.enter_context(tc.tile_pool(name="mm", bufs=3, space="PSUM"))
        py_pool = ctx.enter_context(tc.tile_pool(name="py", bufs=2, space="PSUM"))
        pe_pool = ctx.enter_context(tc.tile_pool(name="pe", bufs=4))
        rr_pool = ctx.enter_context(tc.tile_pool(name="rr", bufs=2))
        ob_pool = ctx.enter_context(tc.tile_pool(name="ob", bufs=3))

        # ---------------- phase 1: projections ----------------
        # qT / kT  (feature-on-partition): psum[o128, t512]
        for i in range(NC4):
            for ch in range(2):
                tsl = slice(ch * 512, ch * 512 + 512)
                psq = mm_pool.tile([128, 512], F32, tag="mm", name=f"psq{i}{ch}")
                for c in range(NC4):
                    nc.tensor.matmul(
                        psq,
                        wq[c][:, i * 128:(i + 1) * 128],
                        xt[c][:, tsl],
                        start=(c == 0), stop=(c == NC4 - 1))
                nc.scalar.activation(out=qT[i][:, tsl], in_=psq, func=IDENT,
                                     bias=bq_sb[:, i:i + 1], scale=1.0)
                psk = mm_pool.tile([128, 512], F32, tag="mm", name=f"psk{i}{ch}")
                for c in range(NC4):
                    nc.tensor.matmul(
                        psk,
                        wk[c][:, i * 128:(i + 1) * 128],
                        xt[c][:, tsl],
                        start=(c == 0), stop=(c == NC4 - 1))
                nc.vector.tensor_scalar_add(kT[i][:, tsl], psk, bk_sb[:, i:i + 1])

        # v natural per t-tile, bias via rank-1 matmul
        for tt in range(NT):
            psv = mm_pool.tile([128, 512], F32, tag="mm", name=f"psv{tt}")
            for c in range(NC4):
                nc.tensor.matmul(
                    psv,
                    xt[c][:, tt * 128:(tt + 1) * 128],
                    wv[c],
                    start=(c == 0), stop=False)
            nc.tensor.matmul(psv, ones_sb[0:1, 0:128],
                             bv_sb, start=False, stop=True)
            psv3 = psv.rearrange("p (h d) -> p h d", h=H)
            nc.vector.tensor_copy(vt[tt][:, :, 0:64], psv3)

        # ---------------- phase 2: attention ----------------
        for h in range(H):
            ht, hr = h // 2, (h % 2) * 64
            lrow = 64
            ysl = slice(0, 64)
            avsl = slice(0, 66)
            psY = py_pool.tile([128, T], F32, tag="py", name=f"psY{h}")
            for p in range(4):
                cols = slice(p * 256, p * 256 + 256)
                for kk in range(p + 1):
                    psS = mm_pool.tile([128, 512], F32, tag="mm",
                                       name=f"psS{h}{p}{kk}")
                    for half in range(2):
                        ki = 2 * kk + half
                        nc.tensor.matmul(
                            psS[:, half * 256:half * 256 + 256],
                            kT[ht][hr:hr + 64, ki * 128:(ki + 1) * 128],
                            qT[ht][hr:hr + 64, cols],
                            start=True, stop=True)
                    pexp = pe_pool.tile([128, 512], F32R, tag="pe",
                                        name=f"pexp{h}{p}{kk}")
                    if general_mask:
                        for half in range(2):
                            ki = 2 * kk + half
                            hsl = slice(half * 256, half * 256 + 256)
                            nc.scalar.activation(
                                out=pexp[:, hsl], in_=psS[:, hsl], func=EXP,
                                bias=km_sb[:, ki:ki + 1], scale=1.0 / math.sqrt(D))
                    else:
                        nc.scalar.activation(out=pexp, in_=psS, func=EXP,
                                             scale=1.0 / math.sqrt(D))
                    if kk == p:  # diagonal pair: block-causal 0/1 mask
                        nc.vector.tensor_mul(pexp, pexp, m01_sb)
                    for half in range(2):
                        ki = 2 * kk + half
                        nc.tensor.matmul(
                            psY[avsl, cols],
                            vt[ki][:, h, :],
                            pexp[:, half * 256:half * 256 + 256],
                            start=(ki == 0), stop=(ki == 2 * p + 1))
            # normalize: r = 1/l, broadcast-DMA over 64 partitions, multiply
            rrow = rr_pool.tile([1, T], F32, tag="rr", name=f"rrow{h}")
            nc.vector.reciprocal(rrow, psY[lrow:lrow + 1, :])
            rrep = rr_pool.tile([64, T], F32, tag="rrep", name=f"rrep{h}")
            nc.gpsimd.partition_broadcast(rrep, rrow)
            nc.vector.tensor_mul(yT[ht][hr:hr + 64, :], psY[ysl, :], rrep)

        # ---------------- phase 3: output projection ----------------
        for tt in range(NT):
            pso = mm_pool.tile([128, 512], F32, tag="mm", name=f"pso{tt}")
            for c in range(NC4):
                nc.tensor.matmul(
                    pso,
                    yT[c][:, tt * 128:(tt + 1) * 128],
                    wp[c],
                    start=(c == 0), stop=False)
            nc.tensor.matmul(pso, ones_sb[0:1, 0:128],
                             bp_sb, start=False, stop=True)
            o_sb = ob_pool.tile([128, 512], F32, tag="ob", name=f"osb{tt}")
            nc.scalar.copy(o_sb, pso)
            nc.sync.dma_start(out=out_ap[tt * 128:(tt + 1) * 128, :], in_=o_sb)

    for f in reversed(frees):
        f()


# ---------------------------------------------------------------------------
# host side
# ---------------------------------------------------------------------------

def _fine_mask01():
    """[128,512] f32: cols 0:256 = [fineM | ones], cols 256:512 = [zeros | fineM].

    fineM[a, b] = 1 if a//8 <= b//8 else 0  (key a, query b within one tile)."""
    a = np.arange(128)
    fine = (a[:, None] // NA <= a[None, :] // NA).astype(np.float32)
    mA = np.concatenate([fine, np.ones((128, 128), np.float32)], axis=1)
    mB = np.concatenate([np.zeros((128, 128), np.float32), fine], axis=1)
    return np.ascontiguousarray(np.concatenate([mA, mB], axis=1))


def make_host_inputs(x, mask, Wq, bq, Wk, bk, Wv, bv, Wp, bp):
    """Returns (shared_inputs, per_core_inputs, general_mask)."""
    f32 = np.float32
    general_mask = not np.all(mask == 1.0)
    shared = {
        "wqT": np.ascontiguousarray(Wq.T.astype(f32)),
        "wkT": np.ascontiguousarray(Wk.T.astype(f32)),
        "wvT": np.ascontiguousarray(Wv.T.astype(f32)),
        "wpT": np.ascontiguousarray(Wp.T.astype(f32)),
        "bq_t": np.ascontiguousarray(bq.astype(f32).reshape(NC4, 128).T),
        "bk_t": np.ascontiguousarray(bk.astype(f32).reshape(NC4, 128).T),
        "bv_row": bv.astype(f32).reshape(1, C).copy(),
        "bp_row": bp.astype(f32).reshape(1, C).copy(),
        "m01": _fine_mask01(),
        "ones_in": np.ones((128, 128), np.float32),
    }
    per_core = []
    for b in range(B):
        d = dict(shared)
        d["xT"] = np.ascontiguousarray(x[b].astype(f32).T)
        if general_mask:
            km = np.where(mask[b] != 0, 0.0, NEG).astype(f32)
            d["kmask"] = np.ascontiguousarray(km.reshape(NT, 128).T)
        per_core.append(d)
    return per_core, general_mask


def build_program(general_mask=False):
    nc = bacc.Bacc("TRN2", target_bir_lowering=False, debug=False, num_devices=1)
    ins = {
        "xT": nc.dram_tensor("xT", [C, T], F32, kind="ExternalInput").ap(),
        "wqT": nc.dram_tensor("wqT", [C, C], F32, kind="ExternalInput").ap(),
        "wkT": nc.dram_tensor("wkT", [C, C], F32, kind="ExternalInput").ap(),
        "wvT": nc.dram_tensor("wvT", [C, C], F32, kind="ExternalInput").ap(),
        "wpT": nc.dram_tensor("wpT", [C, C], F32, kind="ExternalInput").ap(),
        "bq_t": nc.dram_tensor("bq_t", [128, NC4], F32, kind="ExternalInput").ap(),
        "bk_t": nc.dram_tensor("bk_t", [128, NC4], F32, kind="ExternalInput").ap(),
        "bv_row": nc.dram_tensor("bv_row", [1, C], F32, kind="ExternalInput").ap(),
        "bp_row": nc.dram_tensor("bp_row", [1, C], F32, kind="ExternalInput").ap(),
        "m01": nc.dram_tensor("m01", [128, 512], F32, kind="ExternalInput").ap(),
        "ones_in": nc.dram_tensor("ones_in", [128, 128], F32,
                                  kind="ExternalInput").ap(),
    }
    if general_mask:
        ins["kmask"] = nc.dram_tensor("kmask", [128, NT], F32,
                                      kind="ExternalInput").ap()
    out = nc.dram_tensor("out", [T, C], F32, kind="ExternalOutput").ap()
    with tile.TileContext(nc) as tc:
        build_attention(tc, out, ins, general_mask)
    nc.compile()
    return nc


_cached = {}


def get_program(general_mask=False):
    if general_mask not in _cached:
        _cached[general_mask] = build_program(general_mask)
    return _cached[general_mask]


def kernel(x, mask, Wq, bq, Wk, bk, Wv, bv, Wp, bp):
    per_core, general_mask = make_host_inputs(
        x, mask, Wq, bq, Wk, bk, Wv, bv, Wp, bp)
    nc = get_program(general_mask)
    res = run_bass_kernel_spmd(nc, per_core, core_ids=list(range(B)))
    out = np.stack([res.results[b]["out"] for b in range(B)], axis=0)
    return out.astype(np.float32)
